# revision 31
# baseline (speedup 1.0000x reference)
"""HNetLoss on 8 Trainium2 NeuronCores.

Structure of the computation (see reference): the homography maps pixel
(x, y) -> (xp, yp) with denominator den = p5*y + 1 and yp = (p3*y+p4)/den —
both depend ONLY on the row y.  Within a row, xp = alpha*x + beta is affine
in the column index x.  Hence the per-(batch, lane) polynomial fits and
losses are fully determined by three per-(batch, row, lane) reductions over
the columns:

    c  = sum_x [label == lane]
    S1 = sum_x (x-256) * [label == lane]
    S2 = sum_x (x-256)^2 * [label == lane]

The device kernel computes exactly those masked reductions (the only part
that touches the 8 MiB label field); the remaining math is O(B*H*L) scalar
work done on host in float64.

Device strategy (pure data parallelism, batch b -> core b):
  - labels are cast to bf16 (values 0..5, exact) and transposed on host so
    the column index x lies on SBUF partitions: tile [128, 4*256].
  - VectorE builds lane masks 1-4 with tensor_scalar is_equal (bf16, 4x
    DVE mode).  Lane 5 needs no mask: the raw label equals
    sum_l l*mask_l, so its moments are recovered on host from the
    label-moment matmuls and lanes 1-4.
  - TensorE: the 128x128 mask (or label) slice is the STATIONARY operand
    and the tiny [128, 4] column-weight tile [1, x-256, hi((x-256)^2),
    lo((x-256)^2)] is the MOVING operand (hi/lo split keeps the squared
    weights exact in bf16).  Charging only 4 moving rows per matmul keeps
    TensorE far off the critical path.  Each (source, row-half) pair
    accumulates its four x-chunks into a 4-column PSUM strip; all ten
    strips live in one [128, 40] PSUM tile, copied to SBUF and DMA'd out.
  - the input arrives as two DMAs (x-chunks 0-1 via the SP hardware DGE,
    then chunks 2-3 + weights via the GpSimd SWDGE queue) so the first
    half's mask builds overlap the second transfer and the two issue
    paths don't serialize.
"""

import sys

import numpy as np

try:
    import concourse.bass as bass  # noqa: F401
except ModuleNotFoundError:  # pragma: no cover
    sys.path.insert(0, "/opt/trn_rl_repo")

import ml_dtypes

import concourse.bacc as bacc
import concourse.bass as bass
import concourse.mybir as mybir
import concourse.tile as tile

ORDER = 3
N_LANES = 5
EPS_DEN = 1e-5
RIDGE = 1e-6

B, H, W = 8, 256, 512
N_CORES = 8
XC = 256.0  # centering offset for the column weights (keeps bf16 exact)
N_CHUNKS = W // 128
N_SRC = 5  # stationary sources: raw label + masks for lanes 1-4
N_GRP = 2 * N_SRC  # x2 row halves
OUTW = 4 * N_GRP  # psum columns

BF16 = mybir.dt.bfloat16
F32 = mybir.dt.float32


LABW = N_CHUNKS * H  # label columns in the packed input tile
HALFW = LABW // 2  # label columns per input half (chunks 0-1 / 2-3)
INW = LABW + N_CHUNKS * 4  # + 16 weight columns
BW = HALFW + N_CHUNKS * 4  # second-half DMA: the weights + chunks 2-3


def _build_program() -> bass.Bass:
    # Bacc (not raw Bass): its compile() splits multi-wait sync lists into
    # event-semaphore chains — TRN2 allows only 1 wait per instruction, and
    # the Tile kernel-tail Drain alone needs one wait per engine/DMA used.
    nc = bacc.Bacc("TRN2", target_bir_lowering=False)
    inp_d = nc.declare_dram_parameter("inp", [128, INW], BF16, isOutput=False)
    out_d = nc.declare_dram_parameter("moments", [128, OUTW], F32, isOutput=True)

    with tile.TileContext(nc) as tc:
        with (
            tc.tile_pool(name="io", bufs=1) as io_pool,
            tc.tile_pool(name="masks", bufs=2 * (N_SRC - 1) + 3) as mask_pool,
            tc.tile_pool(name="psum", bufs=1, space="PSUM") as psum_pool,
        ):
            # input arrives as two DMAs into separate tiles so the lane masks
            # for x-chunks 0-1 overlap the second half's transfer; the weight
            # columns ride the second DMA (TensorE has slack, VectorE's mask
            # start is gated by the first DMA's semaphore)
            inA = io_pool.tile([128, HALFW], BF16, tag="inA")
            inB = io_pool.tile([128, BW], BF16, tag="inB")
            outb = io_pool.tile([128, OUTW], F32, tag="outb")
            # second half via the GpSimd SWDGE queue: its descriptor
            # generation runs concurrently with the first half's HWDGE issue
            # instead of queueing behind it
            nc.sync.dma_start(inA[:], inp_d[:, :HALFW])
            nc.gpsimd.dma_start(inB[:], inp_d[:, HALFW:])
            wxt = inB[:, HALFW:]
            ps = psum_pool.tile([128, OUTW], F32, tag="ps")

            # stationary sources: the raw label (ready at DMA time, so its
            # matmuls overlap the mask builds) plus lane masks 1-4 built with
            # is_equal.  VectorE is the fast mask engine; GpSimd (otherwise
            # idle) takes the first half's lane-4 mask — VectorE's first-half
            # chain then ends as the second input DMA lands — and the last
            # chunk of the second half's lane-4 mask, shortening the gating
            # VectorE chain by one more chunk.
            labA, labB = inA[:], inB[:, :HALFW]
            stats: dict = {}  # (si, c) -> mask AP covering that x-chunk
            for c in range(N_CHUNKS):
                stats[0, c] = (labA, labB)[c // 2][:, H * (c % 2) : H * (c % 2) + H]

            def build(lane, src_cols, eng, chunks):
                mask = mask_pool.tile([128, H * len(chunks)], BF16)
                eng.tensor_scalar(
                    mask[:], src_cols, float(lane), None, mybir.AluOpType.is_equal
                )
                for i, c in enumerate(chunks):
                    stats[lane, c] = mask[:, H * i : H * i + H]

            # ScalarE (also idle) contributes lane 3's chunk-1 block as two
            # activation passes: q = Square(lab - 3), mask = Relu(1 - q) —
            # exact in bf16 since labels are small integers.  Both functions
            # live in every activation table, so only one early table load.
            qb = mask_pool.tile([128, H], BF16)
            m3c1 = mask_pool.tile([128, H], BF16)
            cm3 = io_pool.tile([128, 1], F32, tag="cm3")
            cm1 = io_pool.tile([128, 1], F32, tag="cm1")
            nc.vector.memset(cm3[:], -3.0)
            nc.vector.memset(cm1[:], -1.0)
            nc.scalar.activation(
                qb[:], labA[:, H:], mybir.ActivationFunctionType.Square, bias=cm3[:]
            )
            nc.scalar.activation(
                m3c1[:], qb[:], mybir.ActivationFunctionType.Relu, bias=1.0, scale=cm1[:]
            )
            stats[3, 1] = m3c1[:]

            build(1, labA, nc.vector, (0, 1))
            build(2, labA, nc.vector, (0, 1))
            build(3, labA[:, :H], nc.vector, (0,))
            build(4, labA, nc.gpsimd, (0, 1))
            for lane in (1, 2, 3):
                build(lane, labB, nc.vector, (2, 3))
            build(4, labB[:, :H], nc.vector, (2,))
            build(4, labB[:, H:], nc.gpsimd, (3,))

            # moments[r, 4*(2*si+h)+i] = sum_x w_i(x) * src_si(x, 128*h+r):
            # src slice [128 x, 128 rows] is the stationary, weights [128, 4]
            # the moving operand, accumulating the 4 x-chunks in PSUM.
            # Groups run back-to-back (one open accumulation at a time: a
            # start=True wipes the whole PSUM bank), ordered so the groups
            # gated on the last-built masks issue last.
            for si in range(N_SRC):
                for h in range(2):
                    g = 2 * si + h
                    for c in range(N_CHUNKS):
                        src = stats[si, c]
                        nc.tensor.matmul(
                            ps[:, 4 * g : 4 * g + 4],
                            src[:, 128 * h : 128 * h + 128],
                            wxt[:, 4 * c : 4 * c + 4],
                            start=(c == 0),
                            stop=(c == N_CHUNKS - 1),
                        )
            # The copy runs on GpSimd: same engine as the trigger, so engine
            # order gives the copy -> DMA-read dependency that Tile cannot
            # infer (the copy is emitted after the prep).
            nc.vector.tensor_copy(outb[:], ps[:])
            nc.sync.dma_start(out_d[:], outb[:])
    nc.compile()
    return nc


def _host_prep(instance_label: np.ndarray):
    """Build per-core input maps: transposed bf16 labels + column weights."""
    lab = np.asarray(instance_label)
    # weights, shared by all cores: wx[p, 4c+j] = w_j(x=128c+p)
    x = np.arange(W, dtype=np.float64)
    xc = x - XC
    xc2 = xc * xc
    hi = xc2.astype(ml_dtypes.bfloat16)
    lo = (xc2 - hi.astype(np.float64)).astype(ml_dtypes.bfloat16)
    wx = np.empty((W, 4), dtype=ml_dtypes.bfloat16)
    wx[:, 0] = 1.0
    wx[:, 1] = xc.astype(ml_dtypes.bfloat16)
    wx[:, 2] = hi
    wx[:, 3] = lo
    wx = wx.reshape(N_CHUNKS, 128, 4).transpose(1, 0, 2).reshape(128, N_CHUNKS * 4)

    in_maps = []
    for b in range(B):
        lt = lab[b].T.astype(ml_dtypes.bfloat16)  # [W, H], values 0..5 exact
        lt = lt.reshape(N_CHUNKS, 128, H).transpose(1, 0, 2).reshape(128, N_CHUNKS * H)
        # [chunks 0-1 | chunks 2-3 | weights] to match the two input DMAs
        packed = np.concatenate([lt, wx], axis=1)
        in_maps.append({"inp": np.ascontiguousarray(packed)})
    return in_maps


def _decode_moments(raw: np.ndarray) -> np.ndarray:
    """Device output [128, OUTW] -> canonical [4, N_LANES*H] (f64).

    raw[r, 4*(2*si+h)+i] = moment-row i of source si at global row 128h+r,
    where source 0 is the raw label and sources 1-4 are lane masks 1-4.
    Lane 5 = (label - sum_{l=1..4} l*mask_l) / 5.
    """
    raw = raw.astype(np.float64).reshape(128, N_GRP, 4)
    per_src = np.empty((N_SRC, 4, H), np.float64)
    for si in range(N_SRC):
        for h in range(2):
            per_src[si, :, 128 * h : 128 * h + 128] = raw[:, 2 * si + h, :].T
    out = np.empty((4, N_LANES * H), np.float64)
    lane5 = per_src[0].copy()
    for lane in range(1, N_SRC):
        out[:, H * (lane - 1) : H * lane] = per_src[lane]
        lane5 -= lane * per_src[lane]
    out[:, H * 4 :] = lane5 / 5.0
    return out


def _finalize(hnet_params: np.ndarray, moments: np.ndarray) -> np.float32:
    """Host-side final math in float64.

    moments: [B, 4, N_LANES*H] f64;
             row j, col H*l+r = sum_x w_j(x) * [label[b,r,x] == l+1]
    """
    p = np.asarray(hnet_params, dtype=np.float64)
    m = moments.astype(np.float64).reshape(B, 4, N_LANES, H).transpose(0, 2, 1, 3)
    c = m[:, :, 0, :]  # [B,L,H]
    S1c = m[:, :, 1, :]
    S2c = m[:, :, 2, :] + m[:, :, 3, :]
    S1 = S1c + XC * c
    S2 = S2c + 2.0 * XC * S1c + XC * XC * c

    r = np.arange(H, dtype=np.float64)
    # match the reference's f32 denominator computation + clamp
    p32 = np.asarray(hnet_params, dtype=np.float32)
    den32 = (p32[:, 5:6] * r.astype(np.float32)[None, :]) + np.float32(1.0)
    den = np.where(np.abs(den32) < EPS_DEN, np.float32(EPS_DEN), den32).astype(
        np.float64
    )
    alpha = p[:, 0:1] / den  # [B,H]
    beta = (p[:, 1:2] * r[None, :] + p[:, 2:3]) / den
    yp = (p[:, 3:4] * r[None, :] + p[:, 4:5]) / den

    al = alpha[:, None, :]  # [B,1,H]
    be = beta[:, None, :]
    Sx = al * S1 + be * c
    Sxx = al * al * S2 + 2 * al * be * S1 + be * be * c

    ypb = yp[:, None, :]  # [B,1,H]
    cnt = c.sum(-1)  # [B,L]
    s = np.stack([(c * ypb**k).sum(-1) for k in range(7)], axis=-1)  # [B,L,7]
    t = np.stack([(Sx * ypb**q).sum(-1) for q in range(4)], axis=-1)  # [B,L,4]
    v = (c * np.abs(den)[:, None, :]).sum(-1)  # [B,L]

    k = ORDER + 1
    A0 = np.empty((B, N_LANES, k, k))
    for i in range(k):
        for j in range(k):
            A0[:, :, i, j] = s[:, :, 6 - i - j]
    rhs = np.stack([t[:, :, 3 - i] for i in range(k)], axis=-1)  # [B,L,4]
    A = A0 + RIDGE * np.eye(k)
    w = np.linalg.solve(A, rhs[..., None])[..., 0]  # [B,L,4]

    xpred = sum(w[:, :, i, None] * ypb ** (3 - i) for i in range(k))  # [B,L,H]
    rss = (Sxx - 2 * xpred * Sx + xpred * xpred * c).sum(-1)  # [B,L]

    cnt_safe = np.maximum(cnt, 1.0)
    lane_loss = (rss / cnt_safe) * (v / cnt_safe)
    valid = (cnt >= ORDER + 1).astype(np.float64)
    nv = valid.sum()
    loss = (valid * lane_loss).sum() / max(nv, 1.0) if nv > 0 else 0.0
    return np.float32(loss)


def _run_device(in_maps, trace: bool = False, trace_cores=None):
    from concourse import bass_utils

    nc = _build_program()
    res = bass_utils.run_bass_kernel_spmd(
        nc,
        in_maps,
        core_ids=list(range(N_CORES)),
        trace=trace,
        trace_cores=trace_cores,
    )
    return res


def kernel(hnet_params: np.ndarray, instance_label: np.ndarray) -> np.ndarray:
    in_maps = _host_prep(instance_label)
    res = _run_device(in_maps)
    moments = np.stack(
        [_decode_moments(np.asarray(res.results[b]["moments"])) for b in range(B)]
    )
    return _finalize(hnet_params, moments)


if __name__ == "__main__":
    # quick CoreSim correctness check against a numpy golden model
    from concourse.bass_interp import CoreSim

    rng = np.random.default_rng(0)
    lab_full = rng.integers(0, 6, size=(B, H, W)).astype(np.int64)
    in_maps = _host_prep(lab_full)

    nc = _build_program()
    sim = CoreSim(nc)
    sim.tensor("inp")[:] = in_maps[0]["inp"]
    sim.simulate()
    mom = _decode_moments(np.asarray(sim.tensor("moments")))

    # golden for batch 0
    x = np.arange(W, dtype=np.float64)
    xc = x - XC
    golden = np.zeros((4, N_LANES * H))
    hi = (xc * xc).astype(ml_dtypes.bfloat16).astype(np.float64)
    lo = (xc * xc) - hi
    for lane in range(N_LANES):
        msk = lab_full[0] == (lane + 1)  # [H, W]
        golden[0, H * lane : H * (lane + 1)] = msk.sum(1)
        golden[1, H * lane : H * (lane + 1)] = (msk * xc).sum(1)
        golden[2, H * lane : H * (lane + 1)] = (msk * hi).sum(1)
        golden[3, H * lane : H * (lane + 1)] = (msk * lo).sum(1)
    err = np.abs(mom - golden)
    rel = err.max() / max(np.abs(golden).max(), 1)
    print("max abs err:", err.max(), "max rel:", rel)
    assert rel < 1e-5, "CoreSim mismatch"
    print("CoreSim moments check PASSED")


# revision 32
# speedup vs baseline: 1.1649x; 1.1649x over previous
"""HNetLoss on 8 Trainium2 NeuronCores.

Structure of the computation (see reference): the homography maps pixel
(x, y) -> (xp, yp) with denominator den = p5*y + 1 and yp = (p3*y+p4)/den —
both depend ONLY on the row y.  Within a row, xp = alpha*x + beta is affine
in the column index x.  Hence the per-(batch, lane) polynomial fits and
losses are fully determined by three per-(batch, row, lane) reductions over
the columns:

    c  = sum_x [label == lane]
    S1 = sum_x (x-256) * [label == lane]
    S2 = sum_x (x-256)^2 * [label == lane]

The device kernel computes exactly those masked reductions (the only part
that touches the 8 MiB label field); the remaining math is O(B*H*L) scalar
work done on host in float64.

Device strategy (pure data parallelism, batch b -> core b):
  - labels are cast to bf16 (values 0..5, exact) and transposed on host so
    the column index x lies on SBUF partitions: tile [128, 4*256].
  - VectorE builds lane masks 1-4 with tensor_scalar is_equal (bf16, 4x
    DVE mode).  Lane 5 needs no mask: the raw label equals
    sum_l l*mask_l, so its moments are recovered on host from the
    label-moment matmuls and lanes 1-4.
  - TensorE: the 128x128 mask (or label) slice is the STATIONARY operand
    and the tiny [128, 4] column-weight tile [1, x-256, hi((x-256)^2),
    lo((x-256)^2)] is the MOVING operand (hi/lo split keeps the squared
    weights exact in bf16).  Charging only 4 moving rows per matmul keeps
    TensorE far off the critical path.  Each (source, row-half) pair
    accumulates its four x-chunks into a 4-column PSUM strip; all ten
    strips live in one [128, 40] PSUM tile, copied to SBUF and DMA'd out.
  - the input arrives as two DMAs (x-chunks 0-1 via the SP hardware DGE,
    then chunks 2-3 + weights via the GpSimd SWDGE queue) so the first
    half's mask builds overlap the second transfer and the two issue
    paths don't serialize.
"""

import sys

import numpy as np

try:
    import concourse.bass as bass  # noqa: F401
except ModuleNotFoundError:  # pragma: no cover
    sys.path.insert(0, "/opt/trn_rl_repo")

import ml_dtypes

import concourse.bacc as bacc
import concourse.bass as bass
import concourse.mybir as mybir
import concourse.tile as tile

ORDER = 3
N_LANES = 5
EPS_DEN = 1e-5
RIDGE = 1e-6

B, H, W = 8, 256, 512
N_CORES = 8
XC = 256.0  # centering offset for the column weights (keeps bf16 exact)
N_CHUNKS = W // 128
N_SRC = 5  # stationary sources: raw label + masks for lanes 1-4
N_GRP = 2 * N_SRC  # x2 row halves
OUTW = 4 * N_GRP  # psum columns

BF16 = mybir.dt.bfloat16
F32 = mybir.dt.float32


LABW = N_CHUNKS * H  # label columns in the packed input tile
HALFW = LABW // 2  # label columns per input half (chunks 0-1 / 2-3)
INW = LABW + N_CHUNKS * 4  # + 16 weight columns
BW = HALFW + N_CHUNKS * 4  # second-half DMA: the weights + chunks 2-3


def _build_program() -> bass.Bass:
    # Bacc (not raw Bass): its compile() splits multi-wait sync lists into
    # event-semaphore chains — TRN2 allows only 1 wait per instruction, and
    # the Tile kernel-tail Drain alone needs one wait per engine/DMA used.
    nc = bacc.Bacc("TRN2", target_bir_lowering=False)
    inp_d = nc.declare_dram_parameter("inp", [128, INW], BF16, isOutput=False)
    out_d = nc.declare_dram_parameter("moments", [128, OUTW], F32, isOutput=True)

    with tile.TileContext(nc) as tc:
        with (
            tc.tile_pool(name="io", bufs=1) as io_pool,
            tc.tile_pool(name="masks", bufs=2 * (N_SRC - 1) + 3) as mask_pool,
            tc.tile_pool(name="psum", bufs=1, space="PSUM") as psum_pool,
        ):
            # input arrives as two DMAs into separate tiles so the lane masks
            # for x-chunks 0-1 overlap the second half's transfer; the weight
            # columns ride the second DMA (TensorE has slack, VectorE's mask
            # start is gated by the first DMA's semaphore)
            inA = io_pool.tile([128, HALFW], BF16, tag="inA")
            inB = io_pool.tile([128, BW], BF16, tag="inB")
            outb = io_pool.tile([128, OUTW], F32, tag="outb")
            # second half via the GpSimd SWDGE queue: its descriptor
            # generation runs concurrently with the first half's HWDGE issue
            # instead of queueing behind it
            nc.sync.dma_start(inA[:], inp_d[:, :HALFW])
            nc.gpsimd.dma_start(inB[:], inp_d[:, HALFW:])
            wxt = inB[:, HALFW:]
            ps = psum_pool.tile([128, OUTW], F32, tag="ps")

            # stationary sources: the raw label (ready at DMA time, so its
            # matmuls overlap the mask builds) plus lane masks 1-4 built with
            # is_equal.  VectorE is the fast mask engine; GpSimd (otherwise
            # idle) takes the first half's lane-4 mask — VectorE's first-half
            # chain then ends as the second input DMA lands — and the last
            # chunk of the second half's lane-4 mask, shortening the gating
            # VectorE chain by one more chunk.
            labA, labB = inA[:], inB[:, :HALFW]
            stats: dict = {}  # (si, c) -> mask AP covering that x-chunk
            for c in range(N_CHUNKS):
                stats[0, c] = (labA, labB)[c // 2][:, H * (c % 2) : H * (c % 2) + H]

            def build(lane, src_cols, eng, chunks):
                mask = mask_pool.tile([128, H * len(chunks)], BF16)
                eng.tensor_scalar(
                    mask[:], src_cols, float(lane), None, mybir.AluOpType.is_equal
                )
                for i, c in enumerate(chunks):
                    stats[lane, c] = mask[:, H * i : H * i + H]

            # ScalarE (also idle) contributes lane 3's chunk-1 block as two
            # activation passes: q = Square(lab - 3), mask = Relu(1 - q) —
            # exact in bf16 since labels are small integers.  Both functions
            # live in every activation table, so only one early table load.
            qb = mask_pool.tile([128, H], BF16)
            m3c1 = mask_pool.tile([128, H], BF16)
            cm3 = io_pool.tile([128, 1], F32, tag="cm3")
            cm1 = io_pool.tile([128, 1], F32, tag="cm1")
            warm = io_pool.tile([128, 1], F32, tag="warm")
            nc.vector.memset(cm3[:], -3.0)
            nc.vector.memset(cm1[:], -1.0)
            # dummy activation with no data deps: hoists the 1.3us activation
            # table load into the input-DMA wait instead of the critical path
            nc.scalar.activation(
                warm[:], cm3[:], mybir.ActivationFunctionType.Square
            )
            nc.scalar.activation(
                qb[:], labA[:, H:], mybir.ActivationFunctionType.Square, bias=cm3[:]
            )
            nc.scalar.activation(
                m3c1[:], qb[:], mybir.ActivationFunctionType.Relu, bias=1.0, scale=cm1[:]
            )
            stats[3, 1] = m3c1[:]

            build(1, labA, nc.vector, (0, 1))
            build(2, labA, nc.vector, (0, 1))
            build(3, labA[:, :H], nc.vector, (0,))
            build(4, labA, nc.gpsimd, (0, 1))
            for lane in (1, 2, 3):
                build(lane, labB, nc.vector, (2, 3))
            build(4, labB[:, :H], nc.vector, (2,))
            build(4, labB[:, H:], nc.gpsimd, (3,))

            # moments[r, 4*(2*si+h)+i] = sum_x w_i(x) * src_si(x, 128*h+r):
            # src slice [128 x, 128 rows] is the stationary, weights [128, 4]
            # the moving operand, accumulating the 4 x-chunks in PSUM.
            # Groups run back-to-back (one open accumulation at a time: a
            # start=True wipes the whole PSUM bank), ordered so the groups
            # gated on the last-built masks issue last.
            for si in range(N_SRC):
                for h in range(2):
                    g = 2 * si + h
                    for c in range(N_CHUNKS):
                        src = stats[si, c]
                        nc.tensor.matmul(
                            ps[:, 4 * g : 4 * g + 4],
                            src[:, 128 * h : 128 * h + 128],
                            wxt[:, 4 * c : 4 * c + 4],
                            start=(c == 0),
                            stop=(c == N_CHUNKS - 1),
                        )
            # The copy runs on GpSimd: same engine as the trigger, so engine
            # order gives the copy -> DMA-read dependency that Tile cannot
            # infer (the copy is emitted after the prep).
            nc.vector.tensor_copy(outb[:], ps[:])
            nc.sync.dma_start(out_d[:], outb[:])
    nc.compile()
    return nc


def _host_prep(instance_label: np.ndarray):
    """Build per-core input maps: transposed bf16 labels + column weights."""
    lab = np.asarray(instance_label)
    # weights, shared by all cores: wx[p, 4c+j] = w_j(x=128c+p)
    x = np.arange(W, dtype=np.float64)
    xc = x - XC
    xc2 = xc * xc
    hi = xc2.astype(ml_dtypes.bfloat16)
    lo = (xc2 - hi.astype(np.float64)).astype(ml_dtypes.bfloat16)
    wx = np.empty((W, 4), dtype=ml_dtypes.bfloat16)
    wx[:, 0] = 1.0
    wx[:, 1] = xc.astype(ml_dtypes.bfloat16)
    wx[:, 2] = hi
    wx[:, 3] = lo
    wx = wx.reshape(N_CHUNKS, 128, 4).transpose(1, 0, 2).reshape(128, N_CHUNKS * 4)

    in_maps = []
    for b in range(B):
        lt = lab[b].T.astype(ml_dtypes.bfloat16)  # [W, H], values 0..5 exact
        lt = lt.reshape(N_CHUNKS, 128, H).transpose(1, 0, 2).reshape(128, N_CHUNKS * H)
        # [chunks 0-1 | chunks 2-3 | weights] to match the two input DMAs
        packed = np.concatenate([lt, wx], axis=1)
        in_maps.append({"inp": np.ascontiguousarray(packed)})
    return in_maps


def _decode_moments(raw: np.ndarray) -> np.ndarray:
    """Device output [128, OUTW] -> canonical [4, N_LANES*H] (f64).

    raw[r, 4*(2*si+h)+i] = moment-row i of source si at global row 128h+r,
    where source 0 is the raw label and sources 1-4 are lane masks 1-4.
    Lane 5 = (label - sum_{l=1..4} l*mask_l) / 5.
    """
    raw = raw.astype(np.float64).reshape(128, N_GRP, 4)
    per_src = np.empty((N_SRC, 4, H), np.float64)
    for si in range(N_SRC):
        for h in range(2):
            per_src[si, :, 128 * h : 128 * h + 128] = raw[:, 2 * si + h, :].T
    out = np.empty((4, N_LANES * H), np.float64)
    lane5 = per_src[0].copy()
    for lane in range(1, N_SRC):
        out[:, H * (lane - 1) : H * lane] = per_src[lane]
        lane5 -= lane * per_src[lane]
    out[:, H * 4 :] = lane5 / 5.0
    return out


def _finalize(hnet_params: np.ndarray, moments: np.ndarray) -> np.float32:
    """Host-side final math in float64.

    moments: [B, 4, N_LANES*H] f64;
             row j, col H*l+r = sum_x w_j(x) * [label[b,r,x] == l+1]
    """
    p = np.asarray(hnet_params, dtype=np.float64)
    m = moments.astype(np.float64).reshape(B, 4, N_LANES, H).transpose(0, 2, 1, 3)
    c = m[:, :, 0, :]  # [B,L,H]
    S1c = m[:, :, 1, :]
    S2c = m[:, :, 2, :] + m[:, :, 3, :]
    S1 = S1c + XC * c
    S2 = S2c + 2.0 * XC * S1c + XC * XC * c

    r = np.arange(H, dtype=np.float64)
    # match the reference's f32 denominator computation + clamp
    p32 = np.asarray(hnet_params, dtype=np.float32)
    den32 = (p32[:, 5:6] * r.astype(np.float32)[None, :]) + np.float32(1.0)
    den = np.where(np.abs(den32) < EPS_DEN, np.float32(EPS_DEN), den32).astype(
        np.float64
    )
    alpha = p[:, 0:1] / den  # [B,H]
    beta = (p[:, 1:2] * r[None, :] + p[:, 2:3]) / den
    yp = (p[:, 3:4] * r[None, :] + p[:, 4:5]) / den

    al = alpha[:, None, :]  # [B,1,H]
    be = beta[:, None, :]
    Sx = al * S1 + be * c
    Sxx = al * al * S2 + 2 * al * be * S1 + be * be * c

    ypb = yp[:, None, :]  # [B,1,H]
    cnt = c.sum(-1)  # [B,L]
    s = np.stack([(c * ypb**k).sum(-1) for k in range(7)], axis=-1)  # [B,L,7]
    t = np.stack([(Sx * ypb**q).sum(-1) for q in range(4)], axis=-1)  # [B,L,4]
    v = (c * np.abs(den)[:, None, :]).sum(-1)  # [B,L]

    k = ORDER + 1
    A0 = np.empty((B, N_LANES, k, k))
    for i in range(k):
        for j in range(k):
            A0[:, :, i, j] = s[:, :, 6 - i - j]
    rhs = np.stack([t[:, :, 3 - i] for i in range(k)], axis=-1)  # [B,L,4]
    A = A0 + RIDGE * np.eye(k)
    w = np.linalg.solve(A, rhs[..., None])[..., 0]  # [B,L,4]

    xpred = sum(w[:, :, i, None] * ypb ** (3 - i) for i in range(k))  # [B,L,H]
    rss = (Sxx - 2 * xpred * Sx + xpred * xpred * c).sum(-1)  # [B,L]

    cnt_safe = np.maximum(cnt, 1.0)
    lane_loss = (rss / cnt_safe) * (v / cnt_safe)
    valid = (cnt >= ORDER + 1).astype(np.float64)
    nv = valid.sum()
    loss = (valid * lane_loss).sum() / max(nv, 1.0) if nv > 0 else 0.0
    return np.float32(loss)


def _run_device(in_maps, trace: bool = False, trace_cores=None):
    from concourse import bass_utils

    nc = _build_program()
    res = bass_utils.run_bass_kernel_spmd(
        nc,
        in_maps,
        core_ids=list(range(N_CORES)),
        trace=trace,
        trace_cores=trace_cores,
    )
    return res


def kernel(hnet_params: np.ndarray, instance_label: np.ndarray) -> np.ndarray:
    in_maps = _host_prep(instance_label)
    res = _run_device(in_maps)
    moments = np.stack(
        [_decode_moments(np.asarray(res.results[b]["moments"])) for b in range(B)]
    )
    return _finalize(hnet_params, moments)


if __name__ == "__main__":
    # quick CoreSim correctness check against a numpy golden model
    from concourse.bass_interp import CoreSim

    rng = np.random.default_rng(0)
    lab_full = rng.integers(0, 6, size=(B, H, W)).astype(np.int64)
    in_maps = _host_prep(lab_full)

    nc = _build_program()
    sim = CoreSim(nc)
    sim.tensor("inp")[:] = in_maps[0]["inp"]
    sim.simulate()
    mom = _decode_moments(np.asarray(sim.tensor("moments")))

    # golden for batch 0
    x = np.arange(W, dtype=np.float64)
    xc = x - XC
    golden = np.zeros((4, N_LANES * H))
    hi = (xc * xc).astype(ml_dtypes.bfloat16).astype(np.float64)
    lo = (xc * xc) - hi
    for lane in range(N_LANES):
        msk = lab_full[0] == (lane + 1)  # [H, W]
        golden[0, H * lane : H * (lane + 1)] = msk.sum(1)
        golden[1, H * lane : H * (lane + 1)] = (msk * xc).sum(1)
        golden[2, H * lane : H * (lane + 1)] = (msk * hi).sum(1)
        golden[3, H * lane : H * (lane + 1)] = (msk * lo).sum(1)
    err = np.abs(mom - golden)
    rel = err.max() / max(np.abs(golden).max(), 1)
    print("max abs err:", err.max(), "max rel:", rel)
    assert rel < 1e-5, "CoreSim mismatch"
    print("CoreSim moments check PASSED")


# revision 36
# speedup vs baseline: 1.1793x; 1.0123x over previous
"""HNetLoss on 8 Trainium2 NeuronCores.

Structure of the computation (see reference): the homography maps pixel
(x, y) -> (xp, yp) with denominator den = p5*y + 1 and yp = (p3*y+p4)/den —
both depend ONLY on the row y.  Within a row, xp = alpha*x + beta is affine
in the column index x.  Hence the per-(batch, lane) polynomial fits and
losses are fully determined by three per-(batch, row, lane) reductions over
the columns:

    c  = sum_x [label == lane]
    S1 = sum_x (x-256) * [label == lane]
    S2 = sum_x (x-256)^2 * [label == lane]

The device kernel computes exactly those masked reductions (the only part
that touches the 8 MiB label field); the remaining math is O(B*H*L) scalar
work done on host in float64.

Device strategy (pure data parallelism, batch b -> core b):
  - labels are cast to bf16 (values 0..5, exact) and transposed on host so
    the column index x lies on SBUF partitions: tile [128, 4*256].
  - VectorE builds lane masks 1-4 with tensor_scalar is_equal (bf16, 4x
    DVE mode).  Lane 5 needs no mask: the raw label equals
    sum_l l*mask_l, so its moments are recovered on host from the
    label-moment matmuls and lanes 1-4.
  - TensorE: the 128x128 mask (or label) slice is the STATIONARY operand
    and the tiny [128, 4] column-weight tile [1, x-256, hi((x-256)^2),
    lo((x-256)^2)] is the MOVING operand (hi/lo split keeps the squared
    weights exact in bf16).  Charging only 4 moving rows per matmul keeps
    TensorE far off the critical path.  Each (source, row-half) pair
    accumulates its four x-chunks into a 4-column PSUM strip; all ten
    strips live in one [128, 40] PSUM tile, copied to SBUF and DMA'd out.
  - the input arrives as two DMAs (x-chunks 0-1 via the SP hardware DGE,
    then chunks 2-3 + weights via the GpSimd SWDGE queue) so the first
    half's mask builds overlap the second transfer and the two issue
    paths don't serialize.
"""

import sys

import numpy as np

try:
    import concourse.bass as bass  # noqa: F401
except ModuleNotFoundError:  # pragma: no cover
    sys.path.insert(0, "/opt/trn_rl_repo")

import ml_dtypes

import concourse.bacc as bacc
import concourse.bass as bass
import concourse.mybir as mybir
import concourse.tile as tile

ORDER = 3
N_LANES = 5
EPS_DEN = 1e-5
RIDGE = 1e-6

B, H, W = 8, 256, 512
N_CORES = 8
XC = 256.0  # centering offset for the column weights (keeps bf16 exact)
N_CHUNKS = W // 128
N_SRC = 5  # stationary sources: raw label + masks for lanes 1-4
N_GRP = 2 * N_SRC  # x2 row halves
OUTW = 4 * N_GRP  # psum columns

BF16 = mybir.dt.bfloat16
F32 = mybir.dt.float32


LABW = N_CHUNKS * H  # label columns in the packed input tile
HALFW = LABW // 2  # label columns per input half (chunks 0-1 / 2-3)
INW = LABW + N_CHUNKS * 4  # + 16 weight columns
BW = HALFW + N_CHUNKS * 4  # second-half DMA: the weights + chunks 2-3


def _build_program() -> bass.Bass:
    # Bacc (not raw Bass): its compile() splits multi-wait sync lists into
    # event-semaphore chains — TRN2 allows only 1 wait per instruction, and
    # the Tile kernel-tail Drain alone needs one wait per engine/DMA used.
    nc = bacc.Bacc("TRN2", target_bir_lowering=False)
    inp_d = nc.declare_dram_parameter("inp", [128, INW], BF16, isOutput=False)
    out_d = nc.declare_dram_parameter("moments", [128, OUTW], F32, isOutput=True)

    with tile.TileContext(nc) as tc:
        with (
            tc.tile_pool(name="io", bufs=1) as io_pool,
            tc.tile_pool(name="masks", bufs=2 * (N_SRC - 1) + 3) as mask_pool,
            tc.tile_pool(name="psum", bufs=1, space="PSUM") as psum_pool,
        ):
            # input arrives as two DMAs into separate tiles so the lane masks
            # for x-chunks 0-1 overlap the second half's transfer; the weight
            # columns ride the second DMA (TensorE has slack, VectorE's mask
            # start is gated by the first DMA's semaphore)
            inA = io_pool.tile([128, HALFW], BF16, tag="inA")
            inB = io_pool.tile([128, BW], BF16, tag="inB")
            outb = io_pool.tile([128, OUTW], F32, tag="outb")
            # second half via the GpSimd SWDGE queue: its descriptor
            # generation runs concurrently with the first half's HWDGE issue
            # instead of queueing behind it
            nc.sync.dma_start(inA[:], inp_d[:, :HALFW])
            nc.gpsimd.dma_start(inB[:], inp_d[:, HALFW:])
            wxt = inB[:, HALFW:]
            ps = psum_pool.tile([128, OUTW], F32, tag="ps")

            # stationary sources: the raw label (ready at DMA time, so its
            # matmuls overlap the elementwise builds) plus the ramp family
            # R_k = max(lab - k, 0), k = 1..4.  Together with the raw label
            # and the host-known per-row totals these span the lane-mask
            # space (R_{k-1} - R_k is the [lab >= k] indicator), and every
            # R_k piece is ONE op on any engine: a chained
            # (subtract, max) tensor_scalar on VectorE/GpSimd (4x DVE mode)
            # or a single Relu activation pass on ScalarE.  The pieces are
            # spread so all three engines finish together.
            labA, labB = inA[:], inB[:, :HALFW]
            stats: dict = {}  # (si, c, h) -> [128, 128] stationary AP
            for c in range(N_CHUNKS):
                t = (labA, labB)[c // 2]
                for h in range(2):
                    o = H * (c % 2) + 128 * h
                    stats[0, c, h] = t[:, o : o + 128]

            def ramp(eng, src_cols, k, pieces):
                """pieces: list of (c, h) half-chunk blocks covered, in order."""
                m = mask_pool.tile([128, 128 * len(pieces)], BF16)
                eng.tensor_scalar(
                    m[:], src_cols, float(k), 0.0,
                    mybir.AluOpType.subtract, mybir.AluOpType.max,
                )
                for i, (c, h) in enumerate(pieces):
                    stats[k, c, h] = m[:, 128 * i : 128 * i + 128]

            def ramp_act(src_cols, k, bias_ap, pieces):
                m = mask_pool.tile([128, 128 * len(pieces)], BF16)
                nc.scalar.activation(
                    m[:], src_cols, mybir.ActivationFunctionType.Relu, bias=bias_ap
                )
                for i, (c, h) in enumerate(pieces):
                    stats[k, c, h] = m[:, 128 * i : 128 * i + 128]

            cm3 = io_pool.tile([128, 1], F32, tag="cm3")
            cm4 = io_pool.tile([128, 1], F32, tag="cm4")
            warm = io_pool.tile([128, 1], F32, tag="warm")
            nc.vector.memset(cm3[:], -3.0)
            nc.vector.memset(cm4[:], -4.0)
            # dummy activation with no data deps: hoists the 1.3us activation
            # table load into the input-DMA wait instead of the critical path
            nc.scalar.activation(warm[:], cm3[:], mybir.ActivationFunctionType.Relu)

            AP4 = [(0, 0), (0, 1), (1, 0), (1, 1)]  # first-half blocks
            BP4 = [(2, 0), (2, 1), (3, 0), (3, 1)]  # second-half blocks
            # first half: VectorE R1, R2; ScalarE R3; GpSimd R4
            ramp(nc.vector, labA, 1, AP4)
            ramp(nc.vector, labA, 2, AP4)
            ramp_act(labA, 3, cm3[:], AP4)
            ramp(nc.gpsimd, labA, 4, AP4)
            # second half: VectorE R1-R3 + R4 chunk 2; GpSimd/ScalarE split
            # the final chunk of R4
            ramp(nc.vector, labB, 1, BP4)
            ramp(nc.vector, labB, 2, BP4)
            ramp(nc.vector, labB, 3, BP4)
            ramp(nc.vector, labB[:, :H], 4, BP4[:2])
            ramp(nc.gpsimd, labB[:, H : H + 128], 4, [BP4[2]])
            ramp_act(labB[:, H + 128 :], 4, cm4[:], [BP4[3]])

            # moments[r, 4*(2*si+h)+i] = sum_x w_i(x) * src_si(x, 128*h+r):
            # src slice [128 x, 128 rows] is the stationary, weights [128, 4]
            # the moving operand, accumulating the 4 x-chunks in PSUM.
            # Groups run back-to-back (one open accumulation at a time: a
            # start=True wipes the whole PSUM bank), ordered so the groups
            # gated on the last-built masks issue last.
            for si in range(N_SRC):
                for h in range(2):
                    g = 2 * si + h
                    for c in range(N_CHUNKS):
                        nc.tensor.matmul(
                            ps[:, 4 * g : 4 * g + 4],
                            stats[si, c, h],
                            wxt[:, 4 * c : 4 * c + 4],
                            start=(c == 0),
                            stop=(c == N_CHUNKS - 1),
                        )
            # The copy runs on GpSimd: same engine as the trigger, so engine
            # order gives the copy -> DMA-read dependency that Tile cannot
            # infer (the copy is emitted after the prep).
            nc.vector.tensor_copy(outb[:], ps[:])
            nc.sync.dma_start(out_d[:], outb[:])
    nc.compile()
    return nc


def _host_prep(instance_label: np.ndarray):
    """Build per-core input maps: transposed bf16 labels + column weights."""
    lab = np.asarray(instance_label)
    # weights, shared by all cores: wx[p, 4c+j] = w_j(x=128c+p)
    x = np.arange(W, dtype=np.float64)
    xc = x - XC
    xc2 = xc * xc
    hi = xc2.astype(ml_dtypes.bfloat16)
    lo = (xc2 - hi.astype(np.float64)).astype(ml_dtypes.bfloat16)
    wx = np.empty((W, 4), dtype=ml_dtypes.bfloat16)
    wx[:, 0] = 1.0
    wx[:, 1] = xc.astype(ml_dtypes.bfloat16)
    wx[:, 2] = hi
    wx[:, 3] = lo
    wx = wx.reshape(N_CHUNKS, 128, 4).transpose(1, 0, 2).reshape(128, N_CHUNKS * 4)

    in_maps = []
    for b in range(B):
        lt = lab[b].T.astype(ml_dtypes.bfloat16)  # [W, H], values 0..5 exact
        lt = lt.reshape(N_CHUNKS, 128, H).transpose(1, 0, 2).reshape(128, N_CHUNKS * H)
        # [chunks 0-1 | chunks 2-3 | weights] to match the two input DMAs
        packed = np.concatenate([lt, wx], axis=1)
        in_maps.append({"inp": np.ascontiguousarray(packed)})
    return in_maps


def _decode_moments(raw: np.ndarray) -> np.ndarray:
    """Device output [128, OUTW] -> canonical [4, N_LANES*H] (f64).

    raw[r, 4*(2*si+h)+i] = moment-row i of source si at global row 128h+r,
    where source 0 is the raw label and sources 1-4 are the ramps
    R_k = max(lab-k, 0).  Second differences of the ramp family recover
    the per-lane masks: with R_0 = lab and R_5 = 0,
    mask_l = R_{l-1} - 2*R_l + R_{l+1}.
    """
    raw = raw.astype(np.float64).reshape(128, N_GRP, 4)
    per_src = np.empty((N_SRC + 2, 4, H), np.float64)  # R_0..R_4 then R_5=0
    for si in range(N_SRC):
        for h in range(2):
            per_src[si, :, 128 * h : 128 * h + 128] = raw[:, 2 * si + h, :].T
    per_src[N_SRC:] = 0.0
    out = np.empty((4, N_LANES * H), np.float64)
    for lane in range(1, N_LANES + 1):
        out[:, H * (lane - 1) : H * lane] = (
            per_src[lane - 1] - 2.0 * per_src[lane] + per_src[lane + 1]
        )
    return out


def _finalize(hnet_params: np.ndarray, moments: np.ndarray) -> np.float32:
    """Host-side final math in float64.

    moments: [B, 4, N_LANES*H] f64;
             row j, col H*l+r = sum_x w_j(x) * [label[b,r,x] == l+1]
    """
    p = np.asarray(hnet_params, dtype=np.float64)
    m = moments.astype(np.float64).reshape(B, 4, N_LANES, H).transpose(0, 2, 1, 3)
    c = m[:, :, 0, :]  # [B,L,H]
    S1c = m[:, :, 1, :]
    S2c = m[:, :, 2, :] + m[:, :, 3, :]
    S1 = S1c + XC * c
    S2 = S2c + 2.0 * XC * S1c + XC * XC * c

    r = np.arange(H, dtype=np.float64)
    # match the reference's f32 denominator computation + clamp
    p32 = np.asarray(hnet_params, dtype=np.float32)
    den32 = (p32[:, 5:6] * r.astype(np.float32)[None, :]) + np.float32(1.0)
    den = np.where(np.abs(den32) < EPS_DEN, np.float32(EPS_DEN), den32).astype(
        np.float64
    )
    alpha = p[:, 0:1] / den  # [B,H]
    beta = (p[:, 1:2] * r[None, :] + p[:, 2:3]) / den
    yp = (p[:, 3:4] * r[None, :] + p[:, 4:5]) / den

    al = alpha[:, None, :]  # [B,1,H]
    be = beta[:, None, :]
    Sx = al * S1 + be * c
    Sxx = al * al * S2 + 2 * al * be * S1 + be * be * c

    ypb = yp[:, None, :]  # [B,1,H]
    cnt = c.sum(-1)  # [B,L]
    s = np.stack([(c * ypb**k).sum(-1) for k in range(7)], axis=-1)  # [B,L,7]
    t = np.stack([(Sx * ypb**q).sum(-1) for q in range(4)], axis=-1)  # [B,L,4]
    v = (c * np.abs(den)[:, None, :]).sum(-1)  # [B,L]

    k = ORDER + 1
    A0 = np.empty((B, N_LANES, k, k))
    for i in range(k):
        for j in range(k):
            A0[:, :, i, j] = s[:, :, 6 - i - j]
    rhs = np.stack([t[:, :, 3 - i] for i in range(k)], axis=-1)  # [B,L,4]
    A = A0 + RIDGE * np.eye(k)
    w = np.linalg.solve(A, rhs[..., None])[..., 0]  # [B,L,4]

    xpred = sum(w[:, :, i, None] * ypb ** (3 - i) for i in range(k))  # [B,L,H]
    rss = (Sxx - 2 * xpred * Sx + xpred * xpred * c).sum(-1)  # [B,L]

    cnt_safe = np.maximum(cnt, 1.0)
    lane_loss = (rss / cnt_safe) * (v / cnt_safe)
    valid = (cnt >= ORDER + 1).astype(np.float64)
    nv = valid.sum()
    loss = (valid * lane_loss).sum() / max(nv, 1.0) if nv > 0 else 0.0
    return np.float32(loss)


def _run_device(in_maps, trace: bool = False, trace_cores=None):
    from concourse import bass_utils

    nc = _build_program()
    res = bass_utils.run_bass_kernel_spmd(
        nc,
        in_maps,
        core_ids=list(range(N_CORES)),
        trace=trace,
        trace_cores=trace_cores,
    )
    return res


def kernel(hnet_params: np.ndarray, instance_label: np.ndarray) -> np.ndarray:
    in_maps = _host_prep(instance_label)
    res = _run_device(in_maps)
    moments = np.stack(
        [_decode_moments(np.asarray(res.results[b]["moments"])) for b in range(B)]
    )
    return _finalize(hnet_params, moments)


if __name__ == "__main__":
    # quick CoreSim correctness check against a numpy golden model
    from concourse.bass_interp import CoreSim

    rng = np.random.default_rng(0)
    lab_full = rng.integers(0, 6, size=(B, H, W)).astype(np.int64)
    in_maps = _host_prep(lab_full)

    nc = _build_program()
    sim = CoreSim(nc)
    sim.tensor("inp")[:] = in_maps[0]["inp"]
    sim.simulate()
    mom = _decode_moments(np.asarray(sim.tensor("moments")))

    # golden for batch 0
    x = np.arange(W, dtype=np.float64)
    xc = x - XC
    golden = np.zeros((4, N_LANES * H))
    hi = (xc * xc).astype(ml_dtypes.bfloat16).astype(np.float64)
    lo = (xc * xc) - hi
    for lane in range(N_LANES):
        msk = lab_full[0] == (lane + 1)  # [H, W]
        golden[0, H * lane : H * (lane + 1)] = msk.sum(1)
        golden[1, H * lane : H * (lane + 1)] = (msk * xc).sum(1)
        golden[2, H * lane : H * (lane + 1)] = (msk * hi).sum(1)
        golden[3, H * lane : H * (lane + 1)] = (msk * lo).sum(1)
    err = np.abs(mom - golden)
    rel = err.max() / max(np.abs(golden).max(), 1)
    print("max abs err:", err.max(), "max rel:", rel)
    assert rel < 1e-5, "CoreSim mismatch"
    print("CoreSim moments check PASSED")


# revision 40
# speedup vs baseline: 1.1809x; 1.0014x over previous
"""HNetLoss on 8 Trainium2 NeuronCores.

Structure of the computation (see reference): the homography maps pixel
(x, y) -> (xp, yp) with denominator den = p5*y + 1 and yp = (p3*y+p4)/den —
both depend ONLY on the row y.  Within a row, xp = alpha*x + beta is affine
in the column index x.  Hence the per-(batch, lane) polynomial fits and
losses are fully determined by three per-(batch, row, lane) reductions over
the columns:

    c  = sum_x [label == lane]
    S1 = sum_x (x-256) * [label == lane]
    S2 = sum_x (x-256)^2 * [label == lane]

The device kernel computes exactly those masked reductions (the only part
that touches the 8 MiB label field); the remaining math is O(B*H*L) scalar
work done on host in float64.

Device strategy (pure data parallelism, batch b -> core b):
  - labels are cast to bf16 (values 0..5, exact) and transposed on host so
    the column index x lies on SBUF partitions: tile [128, 4*256].
  - VectorE builds lane masks 1-4 with tensor_scalar is_equal (bf16, 4x
    DVE mode).  Lane 5 needs no mask: the raw label equals
    sum_l l*mask_l, so its moments are recovered on host from the
    label-moment matmuls and lanes 1-4.
  - TensorE: the 128x128 mask (or label) slice is the STATIONARY operand
    and the tiny [128, 4] column-weight tile [1, x-256, hi((x-256)^2),
    lo((x-256)^2)] is the MOVING operand (hi/lo split keeps the squared
    weights exact in bf16).  Charging only 4 moving rows per matmul keeps
    TensorE far off the critical path.  Each (source, row-half) pair
    accumulates its four x-chunks into a 4-column PSUM strip; all ten
    strips live in one [128, 40] PSUM tile, copied to SBUF and DMA'd out.
  - the input arrives as two DMAs (x-chunks 0-1 via the SP hardware DGE,
    then chunks 2-3 + weights via the GpSimd SWDGE queue) so the first
    half's mask builds overlap the second transfer and the two issue
    paths don't serialize.
"""

import sys

import numpy as np

try:
    import concourse.bass as bass  # noqa: F401
except ModuleNotFoundError:  # pragma: no cover
    sys.path.insert(0, "/opt/trn_rl_repo")

import ml_dtypes

import concourse.bacc as bacc
import concourse.bass as bass
import concourse.mybir as mybir
import concourse.tile as tile

ORDER = 3
N_LANES = 5
EPS_DEN = 1e-5
RIDGE = 1e-6

B, H, W = 8, 256, 512
N_CORES = 8
XC = 256.0  # centering offset for the column weights (keeps bf16 exact)
N_CHUNKS = W // 128
N_SRC = 5  # stationary sources: raw label + masks for lanes 1-4
N_GRP = 2 * N_SRC  # x2 row halves
OUTW = 4 * N_GRP  # psum columns

BF16 = mybir.dt.bfloat16
F32 = mybir.dt.float32


LABW = N_CHUNKS * H  # label columns in the packed input tile
HALFW = LABW // 2  # label columns per input half (chunks 0-1 / 2-3)
INW = LABW + N_CHUNKS * 4  # + 16 weight columns
AW = HALFW + N_CHUNKS * 4  # first-half DMA: chunks 0-1 + the weights


def _build_program() -> bass.Bass:
    # Bacc (not raw Bass): its compile() splits multi-wait sync lists into
    # event-semaphore chains — TRN2 allows only 1 wait per instruction, and
    # the Tile kernel-tail Drain alone needs one wait per engine/DMA used.
    nc = bacc.Bacc("TRN2", target_bir_lowering=False)
    inp_d = nc.declare_dram_parameter("inp", [128, INW], BF16, isOutput=False)
    out_d = nc.declare_dram_parameter("moments", [128, OUTW], F32, isOutput=True)

    with tile.TileContext(nc) as tc:
        with (
            tc.tile_pool(name="io", bufs=1) as io_pool,
            tc.tile_pool(name="masks", bufs=2 * (N_SRC - 1) + 3) as mask_pool,
            tc.tile_pool(name="psum", bufs=1, space="PSUM") as psum_pool,
        ):
            # input arrives as two DMAs into separate tiles so the first
            # half's elementwise builds overlap the second transfer; the
            # weight columns ride the FIRST DMA (the second-half chain is
            # gated by the second DMA's semaphore, so its transfer must stay
            # minimal; the first DMA has VectorE slack)
            inA = io_pool.tile([128, AW], BF16, tag="inA")
            inB = io_pool.tile([128, HALFW], BF16, tag="inB")
            outb = io_pool.tile([128, OUTW], F32, tag="outb")
            # second half via the GpSimd SWDGE queue: its descriptor
            # generation runs concurrently with the first half's HWDGE issue
            # instead of queueing behind it
            nc.sync.dma_start(inA[:], inp_d[:, :AW])
            nc.gpsimd.dma_start(inB[:], inp_d[:, AW:])
            wxt = inA[:, HALFW:]
            ps = psum_pool.tile([128, OUTW], F32, tag="ps")

            # stationary sources: the raw label (ready at DMA time, so its
            # matmuls overlap the elementwise builds) plus the ramp family
            # R_k = max(lab - k, 0), k = 1..4.  Together with the raw label
            # and the host-known per-row totals these span the lane-mask
            # space (R_{k-1} - R_k is the [lab >= k] indicator), and every
            # R_k piece is ONE op on any engine: a chained
            # (subtract, max) tensor_scalar on VectorE/GpSimd (4x DVE mode)
            # or a single Relu activation pass on ScalarE.  The pieces are
            # spread so all three engines finish together.
            labA, labB = inA[:, :HALFW], inB[:]
            stats: dict = {}  # (si, c, h) -> [128, 128] stationary AP
            for c in range(N_CHUNKS):
                t = (labA, labB)[c // 2]
                for h in range(2):
                    o = H * (c % 2) + 128 * h
                    stats[0, c, h] = t[:, o : o + 128]

            def ramp(eng, src_cols, k, pieces):
                """pieces: list of (c, h) half-chunk blocks covered, in order."""
                m = mask_pool.tile([128, 128 * len(pieces)], BF16)
                eng.tensor_scalar(
                    m[:], src_cols, float(k), 0.0,
                    mybir.AluOpType.subtract, mybir.AluOpType.max,
                )
                for i, (c, h) in enumerate(pieces):
                    stats[k, c, h] = m[:, 128 * i : 128 * i + 128]

            def ramp_act(src_cols, k, bias_ap, pieces):
                m = mask_pool.tile([128, 128 * len(pieces)], BF16)
                nc.scalar.activation(
                    m[:], src_cols, mybir.ActivationFunctionType.Relu, bias=bias_ap
                )
                for i, (c, h) in enumerate(pieces):
                    stats[k, c, h] = m[:, 128 * i : 128 * i + 128]

            cm3 = io_pool.tile([128, 1], F32, tag="cm3")
            cm4 = io_pool.tile([128, 1], F32, tag="cm4")
            warm = io_pool.tile([128, 1], F32, tag="warm")
            nc.vector.memset(cm3[:], -3.0)
            nc.vector.memset(cm4[:], -4.0)
            # dummy activation with no data deps: hoists the 1.3us activation
            # table load into the input-DMA wait instead of the critical path
            nc.scalar.activation(warm[:], cm3[:], mybir.ActivationFunctionType.Relu)

            AP4 = [(0, 0), (0, 1), (1, 0), (1, 1)]  # first-half blocks
            BP4 = [(2, 0), (2, 1), (3, 0), (3, 1)]  # second-half blocks
            # first half: VectorE R1, R2; ScalarE R3; GpSimd R4
            ramp(nc.vector, labA, 1, AP4)
            ramp(nc.vector, labA, 2, AP4)
            ramp_act(labA, 3, cm3[:], AP4)
            ramp(nc.gpsimd, labA, 4, AP4)
            # second half: VectorE R1-R3 + R4 chunk 2; GpSimd/ScalarE split
            # the final chunk of R4
            ramp(nc.vector, labB, 1, BP4)
            ramp(nc.vector, labB, 2, BP4)
            ramp(nc.vector, labB, 3, BP4)
            ramp(nc.vector, labB[:, :H], 4, BP4[:2])
            ramp(nc.gpsimd, labB[:, H : H + 128], 4, [BP4[2]])
            ramp_act(labB[:, H + 128 :], 4, cm4[:], [BP4[3]])

            # moments[r, 4*(2*si+h)+i] = sum_x w_i(x) * src_si(x, 128*h+r):
            # src slice [128 x, 128 rows] is the stationary, weights [128, 4]
            # the moving operand, accumulating the 4 x-chunks in PSUM.
            # Groups run back-to-back (one open accumulation at a time: a
            # start=True wipes the whole PSUM bank), ordered so the groups
            # gated on the last-built masks issue last.
            for si in range(N_SRC):
                for h in range(2):
                    g = 2 * si + h
                    for c in range(N_CHUNKS):
                        nc.tensor.matmul(
                            ps[:, 4 * g : 4 * g + 4],
                            stats[si, c, h],
                            wxt[:, 4 * c : 4 * c + 4],
                            start=(c == 0),
                            stop=(c == N_CHUNKS - 1),
                        )
            # The copy runs on GpSimd: same engine as the trigger, so engine
            # order gives the copy -> DMA-read dependency that Tile cannot
            # infer (the copy is emitted after the prep).
            nc.vector.tensor_copy(outb[:], ps[:])
            nc.sync.dma_start(out_d[:], outb[:])
    nc.compile()
    return nc


def _host_prep(instance_label: np.ndarray):
    """Build per-core input maps: transposed bf16 labels + column weights."""
    lab = np.asarray(instance_label)
    # weights, shared by all cores: wx[p, 4c+j] = w_j(x=128c+p)
    x = np.arange(W, dtype=np.float64)
    xc = x - XC
    xc2 = xc * xc
    hi = xc2.astype(ml_dtypes.bfloat16)
    lo = (xc2 - hi.astype(np.float64)).astype(ml_dtypes.bfloat16)
    wx = np.empty((W, 4), dtype=ml_dtypes.bfloat16)
    wx[:, 0] = 1.0
    wx[:, 1] = xc.astype(ml_dtypes.bfloat16)
    wx[:, 2] = hi
    wx[:, 3] = lo
    wx = wx.reshape(N_CHUNKS, 128, 4).transpose(1, 0, 2).reshape(128, N_CHUNKS * 4)

    in_maps = []
    for b in range(B):
        lt = lab[b].T.astype(ml_dtypes.bfloat16)  # [W, H], values 0..5 exact
        lt = lt.reshape(N_CHUNKS, 128, H).transpose(1, 0, 2).reshape(128, N_CHUNKS * H)
        # [chunks 0-1 | weights | chunks 2-3] to match the two input DMAs
        packed = np.concatenate([lt[:, :HALFW], wx, lt[:, HALFW:]], axis=1)
        in_maps.append({"inp": np.ascontiguousarray(packed)})
    return in_maps


def _decode_moments(raw: np.ndarray) -> np.ndarray:
    """Device output [128, OUTW] -> canonical [4, N_LANES*H] (f64).

    raw[r, 4*(2*si+h)+i] = moment-row i of source si at global row 128h+r,
    where source 0 is the raw label and sources 1-4 are the ramps
    R_k = max(lab-k, 0).  Second differences of the ramp family recover
    the per-lane masks: with R_0 = lab and R_5 = 0,
    mask_l = R_{l-1} - 2*R_l + R_{l+1}.
    """
    raw = raw.astype(np.float64).reshape(128, N_GRP, 4)
    per_src = np.empty((N_SRC + 2, 4, H), np.float64)  # R_0..R_4 then R_5=0
    for si in range(N_SRC):
        for h in range(2):
            per_src[si, :, 128 * h : 128 * h + 128] = raw[:, 2 * si + h, :].T
    per_src[N_SRC:] = 0.0
    out = np.empty((4, N_LANES * H), np.float64)
    for lane in range(1, N_LANES + 1):
        out[:, H * (lane - 1) : H * lane] = (
            per_src[lane - 1] - 2.0 * per_src[lane] + per_src[lane + 1]
        )
    return out


def _finalize(hnet_params: np.ndarray, moments: np.ndarray) -> np.float32:
    """Host-side final math in float64.

    moments: [B, 4, N_LANES*H] f64;
             row j, col H*l+r = sum_x w_j(x) * [label[b,r,x] == l+1]
    """
    p = np.asarray(hnet_params, dtype=np.float64)
    m = moments.astype(np.float64).reshape(B, 4, N_LANES, H).transpose(0, 2, 1, 3)
    c = m[:, :, 0, :]  # [B,L,H]
    S1c = m[:, :, 1, :]
    S2c = m[:, :, 2, :] + m[:, :, 3, :]
    S1 = S1c + XC * c
    S2 = S2c + 2.0 * XC * S1c + XC * XC * c

    r = np.arange(H, dtype=np.float64)
    # match the reference's f32 denominator computation + clamp
    p32 = np.asarray(hnet_params, dtype=np.float32)
    den32 = (p32[:, 5:6] * r.astype(np.float32)[None, :]) + np.float32(1.0)
    den = np.where(np.abs(den32) < EPS_DEN, np.float32(EPS_DEN), den32).astype(
        np.float64
    )
    alpha = p[:, 0:1] / den  # [B,H]
    beta = (p[:, 1:2] * r[None, :] + p[:, 2:3]) / den
    yp = (p[:, 3:4] * r[None, :] + p[:, 4:5]) / den

    al = alpha[:, None, :]  # [B,1,H]
    be = beta[:, None, :]
    Sx = al * S1 + be * c
    Sxx = al * al * S2 + 2 * al * be * S1 + be * be * c

    ypb = yp[:, None, :]  # [B,1,H]
    cnt = c.sum(-1)  # [B,L]
    s = np.stack([(c * ypb**k).sum(-1) for k in range(7)], axis=-1)  # [B,L,7]
    t = np.stack([(Sx * ypb**q).sum(-1) for q in range(4)], axis=-1)  # [B,L,4]
    v = (c * np.abs(den)[:, None, :]).sum(-1)  # [B,L]

    k = ORDER + 1
    A0 = np.empty((B, N_LANES, k, k))
    for i in range(k):
        for j in range(k):
            A0[:, :, i, j] = s[:, :, 6 - i - j]
    rhs = np.stack([t[:, :, 3 - i] for i in range(k)], axis=-1)  # [B,L,4]
    A = A0 + RIDGE * np.eye(k)
    w = np.linalg.solve(A, rhs[..., None])[..., 0]  # [B,L,4]

    xpred = sum(w[:, :, i, None] * ypb ** (3 - i) for i in range(k))  # [B,L,H]
    rss = (Sxx - 2 * xpred * Sx + xpred * xpred * c).sum(-1)  # [B,L]

    cnt_safe = np.maximum(cnt, 1.0)
    lane_loss = (rss / cnt_safe) * (v / cnt_safe)
    valid = (cnt >= ORDER + 1).astype(np.float64)
    nv = valid.sum()
    loss = (valid * lane_loss).sum() / max(nv, 1.0) if nv > 0 else 0.0
    return np.float32(loss)


def _run_device(in_maps, trace: bool = False, trace_cores=None):
    from concourse import bass_utils

    nc = _build_program()
    res = bass_utils.run_bass_kernel_spmd(
        nc,
        in_maps,
        core_ids=list(range(N_CORES)),
        trace=trace,
        trace_cores=trace_cores,
    )
    return res


def kernel(hnet_params: np.ndarray, instance_label: np.ndarray) -> np.ndarray:
    in_maps = _host_prep(instance_label)
    res = _run_device(in_maps)
    moments = np.stack(
        [_decode_moments(np.asarray(res.results[b]["moments"])) for b in range(B)]
    )
    return _finalize(hnet_params, moments)


if __name__ == "__main__":
    # quick CoreSim correctness check against a numpy golden model
    from concourse.bass_interp import CoreSim

    rng = np.random.default_rng(0)
    lab_full = rng.integers(0, 6, size=(B, H, W)).astype(np.int64)
    in_maps = _host_prep(lab_full)

    nc = _build_program()
    sim = CoreSim(nc)
    sim.tensor("inp")[:] = in_maps[0]["inp"]
    sim.simulate()
    mom = _decode_moments(np.asarray(sim.tensor("moments")))

    # golden for batch 0
    x = np.arange(W, dtype=np.float64)
    xc = x - XC
    golden = np.zeros((4, N_LANES * H))
    hi = (xc * xc).astype(ml_dtypes.bfloat16).astype(np.float64)
    lo = (xc * xc) - hi
    for lane in range(N_LANES):
        msk = lab_full[0] == (lane + 1)  # [H, W]
        golden[0, H * lane : H * (lane + 1)] = msk.sum(1)
        golden[1, H * lane : H * (lane + 1)] = (msk * xc).sum(1)
        golden[2, H * lane : H * (lane + 1)] = (msk * hi).sum(1)
        golden[3, H * lane : H * (lane + 1)] = (msk * lo).sum(1)
    err = np.abs(mom - golden)
    rel = err.max() / max(np.abs(golden).max(), 1)
    print("max abs err:", err.max(), "max rel:", rel)
    assert rel < 1e-5, "CoreSim mismatch"
    print("CoreSim moments check PASSED")
